# revision 3
# baseline (speedup 1.0000x reference)
"""Trainium2 Bass kernel for nn_AttentionBlock (B=8, L=2048, E=1024, ND=512).

v2 design:
- scores computed TRANSPOSED (kT as stationary): exp(scores^T) is directly the
  lhsT of the attn@v matmul -> zero PE transposes in attention.
- softmax denominator dropped entirely: LN1 is invariant to per-row scaling,
  so dividing by sum(exp) is a mathematical no-op. No row-max subtraction
  either (|scores| < ~40, exp stays in fp32/bf16 range).
- all matmul operands 16-bit (fp16 for x/q/k/ff path, bf16 for exp(scores)
  and v which need >fp16 range); fp32 PSUM accumulation throughout.
- inputs packed into ONE fp16 blob + one small f32 vector tensor per core
  (cuts per-call input re-bind cost: 26 MB f32 -> 13 MB fp16, 11 handles -> 2).
- h round-trips DRAM as fp16 solely to come back via DMA-transpose (xbar),
  removing the FFN's PE-transposes as well.
"""

import math
import sys

if "/opt/trn_rl_repo" not in sys.path:
    sys.path.insert(0, "/opt/trn_rl_repo")

import numpy as np

import concourse.bass as bass
import concourse.tile as tile
from concourse import bacc, mybir
from concourse.bass_utils import run_bass_kernel_spmd

F32 = mybir.dt.float32
F16 = mybir.dt.float16
BF16 = mybir.dt.bfloat16
AF = mybir.ActivationFunctionType
ALU = mybir.AluOpType
AX = mybir.AxisListType

P = 128
E = 1024
ND = 512
F = 2048
L = 2048
LN_EPS = 1e-5
SCALE = math.sqrt(1.0 / E) * 2.0 * math.log(2048)
# Row-uniform scale on the (denominator-free) attention values. LN1 absorbs
# it exactly; it recenters u = exp(scores) @ v into fp32's range so the
# sum-of-squares in LN1 neither overflows (top rows have scores ~50) nor
# sinks below LN_EPS (weakest rows have row-max ~20).
V_ALPHA = 1e-7

EC = E // P  # 8
NDC = ND // P  # 4
FC = F // P  # 16
LT = L // P  # 16
NB = L // 512  # 4

# fp16 blob element offsets (per batch-element x section, then weights)
XT_N = E * L  # 2097152
WQK_N = 16 * P * EC * P  # 2097152
WVT_N = P * EC * ND  # 524288
W1_N = P * NDC * F  # 1048576
W2_N = P * FC * ND  # 1048576
W_N = WQK_N + WVT_N + W1_N + W2_N + 4608  # trailing fp16 vec pack

N_CORES = 8
BC = 8 // N_CORES  # batch elements per core


def _view(ap, off, dims, strides):
    return bass.AP(tensor=ap.tensor, offset=off, ap=[[s, n] for s, n in zip(strides, dims)])


def _bcast(ap, off, n, parts=P):
    return bass.AP(tensor=ap.tensor, offset=off, ap=[[0, parts], [1, n]])


def _layernorm(nc, pool, spool, z, gb, bb, eps_t, out_t):
    """out_t = LN(z) * gb + bb.  z: [P, ND] fp32 (SBUF or PSUM)."""
    nmean = spool.tile([P, 1], F32, tag="nmean")
    nc.vector.reduce_sum(nmean[:], z[:], axis=AX.X)
    nc.vector.tensor_scalar_mul(nmean[:], nmean[:], -1.0 / ND)
    hc = pool.tile([P, ND], F32, tag="ln_hc")
    nc.scalar.activation(hc[:], z[:], AF.Identity, bias=nmean[:])
    sq = pool.tile([P, ND], F32, tag="ln_sq")
    ssq = spool.tile([P, 1], F32, tag="ssq")
    nc.scalar.activation(sq[:], hc[:], AF.Square, accum_out=ssq[:])
    std = spool.tile([P, 1], F32, tag="std")
    nc.scalar.activation(std[:], ssq[:], AF.Sqrt, bias=eps_t[:], scale=1.0 / ND)
    rstd = spool.tile([P, 1], F32, tag="rstd")
    nc.vector.reciprocal(rstd[:], std[:])
    hs = pool.tile([P, ND], F32, tag="ln_hs")
    nc.scalar.activation(hs[:], hc[:], AF.Copy, scale=rstd[:])
    nc.vector.tensor_tensor(hs[:], hs[:], gb[:], ALU.mult)
    nc.vector.tensor_tensor(out_t[:], hs[:], bb[:], ALU.add)


def _kernel(tc, blob, out, bc=BC):
    nc = tc.nc
    XB = 256  # xT streaming block columns
    NXB = L // XB  # 8

    woff = bc * XT_N
    wqk_off = woff
    wvt_off = woff + WQK_N
    w1_off = wvt_off + WVT_N
    w2_off = w1_off + W1_N
    voff = w2_off + W2_N

    from contextlib import ExitStack

    ctx = ExitStack()
    with ctx:
        ps = ctx.enter_context(tc.tile_pool(name="psum", bufs=8, space="PSUM"))
        dram = ctx.enter_context(tc.tile_pool(name="dram", bufs=1, space="DRAM"))
        const = ctx.enter_context(tc.tile_pool(name="const", bufs=1))
        wpool = ctx.enter_context(tc.tile_pool(name="w", bufs=1))

        h_d = dram.tile([L, ND], F16)

        eps_t = const.tile([P, 1], F32)
        nc.vector.memset(eps_t[:], LN_EPS)

        def vec_bcast(off, n):
            stage = const.tile([P, n], F16, tag=f"vstage{off}")
            nc.scalar.dma_start(stage[:], _bcast(blob, voff + off, n))
            t = const.tile([P, n], F32, tag=f"vf32{off}")
            nc.vector.tensor_copy(t[:], stage[:])
            return t

        b2b = vec_bcast(2048, ND)
        g1b = vec_bcast(2560, ND)
        be1b = vec_bcast(3072, ND)
        g2b = vec_bcast(3584, ND)
        be2b = vec_bcast(4096, ND)
        b1s = const.tile([P, FC], F16, tag="b1s")
        nc.scalar.dma_start(b1s[:], _view(blob, voff, [P, FC], [1, P]))
        b1p = const.tile([P, FC], F32)
        nc.vector.tensor_copy(b1p[:], b1s[:])

        # ---- weights (wq/w1/w2 resident; wv/wk are pass-A-scoped) ----
        wq = wpool.tile([P, 8, EC, P], F16, name="wq")
        w1 = wpool.tile([P, NDC, F], F16)
        w2 = wpool.tile([P, FC, ND], F16)

        def wqk_tile(t):
            return _view(blob, wqk_off + t * (P * EC * P), [P, EC, P], [EC * P, P, 1])

        for b in range(bc):
            xt_off = b * XT_N

            def xT_blk(c0, ncols):
                return _view(blob, xt_off + c0, [P, EC, ncols], [L, P * L, 1])

            pH_cm = tc.tile_pool(name="pH", bufs=1)
            pH = pH_cm.__enter__()
            h_sb = pH.tile([P, LT, ND], F16)
            pE_cm = tc.tile_pool(name="pE", bufs=1)
            pE = pE_cm.__enter__()
            pLN_cm = tc.tile_pool(name="pLN", bufs=1)
            pLN = pLN_cm.__enter__()
            xb2_0 = pE.tile([P, EC, 512], F16)
            hT_all = pE.tile([P, NB, NDC, 512], F16)
            kv_cm = tc.tile_pool(name="kv", bufs=1)
            kv = kv_cm.__enter__()
            kT_sb = kv.tile([P, EC, L], F16)  # [e-chunk, s]
            v_sb = kv.tile([P, LT, ND], BF16)  # [s-chunk, nd]
            wkv_cm = tc.tile_pool(name="wkv", bufs=1)
            wkv = wkv_cm.__enter__()
            wv = wkv.tile([P, EC, ND], F16)
            wk = wkv.tile([P, 8, EC, P], F16, name="wk")
            # only wk[0] + the first x block gate the first matmul; queue
            # them ahead of the remaining weights
            nc.sync.dma_start(wk[:, 0], wqk_tile(8))

            # ---------------- pass A: kT and v ----------------
            with tc.tile_pool(name="pA", bufs=2) as pA:
                for xb8 in range(NXB):
                    xb = pA.tile([P, EC, XB], F16, tag="xb")
                    nc.sync.dma_start(xb[:], xT_blk(xb8 * XB, XB))
                    if xb8 == 0:
                        nc.sync.dma_start(
                            wv[:], _view(blob, wvt_off, [P, EC, ND], [EC * ND, ND, 1])
                        )
                        for t in range(1, 8):
                            nc.sync.dma_start(wk[:, t], wqk_tile(8 + t))
                    for t in range(8):
                        pk = ps.tile([P, XB], F32, tag="ps", name="pk")
                        for c in range(EC):
                            nc.tensor.matmul(
                                pk[:],
                                wk[:, t, c, :],
                                xb[:, c, :],
                                start=(c == 0),
                                stop=(c == EC - 1),
                            )
                        nc.vector.tensor_copy(
                            kT_sb[:, t, xb8 * XB : (xb8 + 1) * XB], pk[:]
                        )
                    for j in range(XB // P):
                        pv = ps.tile([P, 512], F32, tag="ps", name="pv")
                        for c in range(EC):
                            nc.tensor.matmul(
                                pv[:],
                                xb[:, c, j * P : (j + 1) * P],
                                wv[:, c, :],
                                start=(c == 0),
                                stop=(c == EC - 1),
                            )
                        nc.scalar.activation(
                            v_sb[:, xb8 * (XB // P) + j, :], pv[:],
                            AF.Copy, scale=V_ALPHA,
                        )
                    if xb8 == 0:
                        # prefetch q weights + pass-B's first x block behind
                        # the first block's compute
                        if b == 0:
                            for t in range(8):
                                nc.scalar.dma_start(wq[:, t], wqk_tile(t))
                        nc.scalar.dma_start(xb2_0[:], xT_blk(0, 512))

            wkv_cm.__exit__(None, None, None)

            # ---------------- pass B: q + attention + LN1 ----------------
            with (
                tc.tile_pool(name="pB", bufs=2) as pB,
                tc.tile_pool(name="pBs", bufs=4) as pBs,
                tc.tile_pool(name="pP", bufs=1) as pP,
            ):
                for lb in range(NB):
                    qt = pB.tile([P, EC, 512], F16, tag="qt")
                    if lb == 0:
                        for t in range(8):
                            pq = ps.tile([P, 512], F32, tag="ps", name="pq5")
                            for c in range(EC):
                                nc.tensor.matmul(
                                    pq[:],
                                    wq[:, t, c, :],
                                    xb2_0[:, c, :],
                                    start=(c == 0),
                                    stop=(c == EC - 1),
                                )
                            nc.vector.tensor_copy(qt[:, t, :], pq[:])
                    else:
                        for half in range(512 // XB):
                            xb = pB.tile([P, EC, XB], F16, tag="xb2", name="xb2")
                            nc.sync.dma_start(
                                xb[:], xT_blk(lb * 512 + half * XB, XB)
                            )
                            for t in range(8):
                                pq = ps.tile([P, XB], F32, tag="ps", name="pq")
                                for c in range(EC):
                                    nc.tensor.matmul(
                                        pq[:],
                                        wq[:, t, c, :],
                                        xb[:, c, :],
                                        start=(c == 0),
                                        stop=(c == EC - 1),
                                    )
                                nc.vector.tensor_copy(
                                    qt[:, t, half * XB : (half + 1) * XB], pq[:]
                                )
                    # scores^T for all 16 s-tiles, exp in place -> pT (bf16)
                    pT = pP.tile([P, LT, 512], BF16, tag="pT")
                    for st in range(LT):
                        pp = ps.tile([P, 512], F32, tag="ps", name="pp")
                        for c in range(EC):
                            nc.tensor.matmul(
                                pp[:],
                                kT_sb[:, c, st * P : (st + 1) * P],
                                qt[:, c, :],
                                start=(c == 0),
                                stop=(c == EC - 1),
                            )
                        nc.scalar.activation(pT[:, st, :], pp[:], AF.Exp)
                    # attn rows (unnormalized; LN1 absorbs the denominator)
                    for j in range(4):
                        po = ps.tile([P, 512], F32, tag="ps", name="po")
                        for st in range(LT):
                            nc.tensor.matmul(
                                po[:],
                                pT[:, st, j * P : (j + 1) * P],
                                v_sb[:, st, :],
                                start=(st == 0),
                                stop=(st == LT - 1),
                            )
                        lt = lb * 4 + j
                        h_t = pB.tile([P, ND], F16, tag="h")
                        _layernorm(nc, pLN, pBs, po, g1b, be1b, eps_t, h_t)
                        nc.vector.tensor_copy(h_sb[:, lt, :], h_t[:])
                        nc.sync.dma_start(h_d[lt * P : (lt + 1) * P, :], h_t[:])
                    # transpose this block's h back in via the xbar while
                    # attention continues on later blocks
                    for c in range(NDC):
                        nc.scalar.dma_start_transpose(
                            hT_all[:, lb, c, :],
                            h_d[lb * 512 : (lb + 1) * 512, c * P : (c + 1) * P],
                        )
                    if b == 0 and lb == 0:
                        # prefetch FFN weights behind attention compute
                        nc.scalar.dma_start(
                            w1[:], _view(blob, w1_off, [P, NDC, F], [NDC * F, F, 1])
                        )
                    if b == 0 and lb == 1:
                        nc.scalar.dma_start(
                            w2[:], _view(blob, w2_off, [P, FC, ND], [FC * ND, ND, 1])
                        )

                kv_cm.__exit__(None, None, None)

                # ---------------- phase 3: FFN + LN2 ----------------
                with (
                    tc.tile_pool(name="p3", bufs=2) as p3,
                    tc.tile_pool(name="p3f", bufs=2) as p3f,
                    tc.tile_pool(name="p3s", bufs=4) as p3s,
                ):
                    for fb in range(NB):
                        hT = p3.tile([P, NDC, 512], F16, tag="hT")
                        for c in range(NDC):
                            nc.scalar.dma_start_transpose(
                                hT[:, c, :],
                                h_d[fb * 512 : (fb + 1) * 512, c * P : (c + 1) * P],
                            )
                        ffT = p3f.tile([P, FC, 512], F16, tag="ffT")
                        for ft in range(FC):
                            pf = ps.tile([P, 512], F32, tag="ps", name="pf")
                            for c in range(NDC):
                                nc.tensor.matmul(
                                    pf[:],
                                    w1[:, c, ft * P : (ft + 1) * P],
                                    hT[:, c, :],
                                    start=(c == 0),
                                    stop=(c == NDC - 1),
                                )
                            nc.scalar.activation(
                                ffT[:, ft, :], pf[:], AF.Relu,
                                bias=b1p[:, ft : ft + 1],
                            )
                        for t4 in range(4):
                            p2o = ps.tile([P, 512], F32, tag="ps", name="p2o")
                            for fc in range(FC):
                                nc.tensor.matmul(
                                    p2o[:],
                                    ffT[:, fc, t4 * P : (t4 + 1) * P],
                                    w2[:, fc, :],
                                    start=(fc == 0),
                                    stop=(fc == FC - 1),
                                )
                            lt = fb * 4 + t4
                            hf = p3.tile([P, ND], F32, tag="hf")
                            nc.vector.tensor_copy(hf[:], h_sb[:, lt, :])
                            z = p3.tile([P, ND], F32, tag="z")
                            nc.vector.tensor_tensor(z[:], p2o[:], hf[:], ALU.add)
                            nc.vector.tensor_tensor(z[:], z[:], b2b[:], ALU.add)
                            o_t = p3.tile([P, ND], F16, tag="o")
                            _layernorm(nc, p3, p3s, z, g2b, be2b, eps_t, o_t)
                            row = b * L + lt * P
                            nc.sync.dma_start(out[row : row + P, :], o_t[:])


def build_program(reps=1, bc=BC):
    nc = bacc.Bacc("TRN2", target_bir_lowering=False, debug=False)
    blob = nc.dram_tensor(
        "blob", [bc * XT_N + W_N], F16, kind="ExternalInput"
    ).ap()
    out = nc.dram_tensor("out", [bc * L, ND], F16, kind="ExternalOutput").ap()
    with tile.TileContext(nc) as tc:
        for _ in range(reps):
            _kernel(tc, blob, out, bc=bc)
    nc.compile()
    return nc


def make_in_maps(x, in_proj_w, w1, b1, w2, b2, g1, be1, g2, be2, bc=BC):
    B = x.shape[0]
    n_cores = B // bc
    wT = np.asarray(in_proj_w, np.float64).T.copy()
    wT[:, :E] *= SCALE
    wqk = np.ascontiguousarray(
        wT[:, : 2 * E].reshape(EC, P, 16, P).transpose(2, 1, 0, 3)
    ).astype(np.float16)
    wvt = (
        np.ascontiguousarray(wT[:, 2 * E :].reshape(EC, P, ND).transpose(1, 0, 2))
    ).astype(np.float16)
    w1t = np.ascontiguousarray(
        np.asarray(w1, np.float32).T.reshape(NDC, P, F).transpose(1, 0, 2)
    ).astype(np.float16)
    w2t = np.ascontiguousarray(
        np.asarray(w2, np.float32).T.reshape(FC, P, ND).transpose(1, 0, 2)
    ).astype(np.float16)
    vecs = np.concatenate(
        [
            np.asarray(b1, np.float32),
            np.asarray(b2, np.float32),
            np.asarray(g1, np.float32),
            np.asarray(be1, np.float32),
            np.asarray(g2, np.float32),
            np.asarray(be2, np.float32),
        ]
    ).astype(np.float16)
    wflat = np.concatenate(
        [wqk.ravel(), wvt.ravel(), w1t.ravel(), w2t.ravel(), vecs]
    )
    xT = np.transpose(np.asarray(x, np.float32), (0, 2, 1)).astype(np.float16)
    maps = []
    for core in range(n_cores):
        xs = xT[core * bc : (core + 1) * bc].ravel()
        maps.append(dict(blob=np.concatenate([xs, wflat])))
    return maps


def kernel(**inputs):
    in_maps = make_in_maps(**inputs)
    nc = build_program()
    res = run_bass_kernel_spmd(nc, in_maps, list(range(len(in_maps))))
    outs = [r["out"].astype(np.float32) for r in res.results]
    B = inputs["x"].shape[0]
    return np.concatenate(outs, axis=0).reshape(B, L, ND)


if __name__ == "__main__":
    nc = build_program()
    print("built ok")
